# revision 13
# baseline (speedup 1.0000x reference)
"""v8: Causal single-head attention (B=4, S=4096, D=1024, H=64) on 8 TRN2 cores.

8-core split: 4 batches x 2 interleaved query-fold roles, 512-row chunks.
vs v2 baseline (36.2us):
- q and v strips in fp8 e3m4 (PE handles e3m4 denormals exactly); k stays
  bf16 so score quantization noise is ~1/sqrt(2) of the all-e3m4 variant.
  W prescaled x32 on host (bf16); scores descaled 1/1024 inside exp.
  DMA 12.6MB -> 8.4MB per core (the baseline was DMA-bound at 34.6us).
- e3m4 strips loaded two-at-a-time (1KB contiguous runs; 512B runs run
  at ~45% bandwidth).
- V^T produced directly: v-strip chunks as stationary (lhsT) against Wv.
- exp split: diag pairs ACT bf16 + DVE mask; local full pairs ACT bf16
  (exact-V bf16 PV); peer full pairs ACT->fp8e4 with DoubleRow PV + a
  dup-ones DR matmul accumulating the denominator; every 3rd full pair
  goes to DVE via 1-instruction Schraudolph (int16 bits of bf16).
- peer K/V exchange via ReduceScatter(add) with send-side role masks.
- host rescue: rare fp8 exp overflow -> non-finite column recomputed
  exactly on host.

Output oT [66, 2048] f32: rows 0:64 numerator^T (x32), row 64 diag+bf16
den, row 65 fp8 den; division on host with den = row64 + row65.
"""

import numpy as np
import ml_dtypes

import concourse.bacc as bacc
import concourse.mybir as mybir
import concourse.tile as tile
from concourse.bass_utils import run_bass_kernel_spmd

B, S, D, H = 4, 4096, 1024, 64
SBLK = 512
NCH = D // 128
QLOC = 2048
NSLOT = QLOC // SBLK
NVB = QLOC // 128

F32 = mybir.dt.float32
BF16 = mybir.dt.bfloat16
E4 = mybir.dt.float8e4
E3 = mybir.dt.float8e3
I16 = mybir.dt.int16
DR = mybir.MatmulPerfMode.DoubleRow

RG_PAIRS = [[0, 1], [2, 3], [4, 5], [6, 7]]

LOG2E = 1.4426950408889634
LN16 = 2.772588722239781
WPRE = 32.0
SCALE = 0.125 / (WPRE * WPRE)
A16 = 128.0 * LOG2E * SCALE
B16 = 127.0 * 128.0 - 5.6 - LN16 * 128.0 * LOG2E


def build_kernel(repeat: int = 1, phase: str = "full", dve_every: int = 3):
    nc = bacc.Bacc("TRN2", target_bir_lowering=False, debug=False, num_devices=8)

    qT = nc.dram_tensor("qT", [D, QLOC], E3, kind="ExternalInput")
    kT = nc.dram_tensor("kT", [D, QLOC], BF16, kind="ExternalInput")
    vT = nc.dram_tensor("vT", [D, QLOC], E3, kind="ExternalInput")
    wqT = nc.dram_tensor("wqT", [D, H], BF16, kind="ExternalInput")
    wkT = nc.dram_tensor("wkT", [D, H], BF16, kind="ExternalInput")
    wvT = nc.dram_tensor("wvT", [D, H], BF16, kind="ExternalInput")
    masks = nc.dram_tensor("masks", [4, 128, SBLK], BF16, kind="ExternalInput")
    rolesel = nc.dram_tensor("rolesel", [128, 2], F32, kind="ExternalInput")
    out = nc.dram_tensor("oT", [H + 2, QLOC], F32, kind="ExternalOutput")

    with tile.TileContext(nc) as tc:
        with (
            tc.tile_pool(name="const", bufs=1) as const_pool,
            tc.tile_pool(name="big", bufs=1) as big_pool,
            tc.tile_pool(name="strips", bufs=3) as strip_pool,
            tc.tile_pool(name="work", bufs=8) as work_pool,
            tc.tile_pool(name="owork", bufs=3) as owork_pool,
            tc.tile_pool(name="pp", bufs=2, space="PSUM") as pp,
            tc.tile_pool(name="ps_sc", bufs=2, space="PSUM") as ps_sc,
            tc.tile_pool(name="ps_o", bufs=1, space="PSUM") as ps_o,
            tc.tile_pool(name="ps_dn", bufs=1, space="PSUM") as ps_dn,
            tc.tile_pool(name="dram", bufs=1, space="DRAM") as dram_pool,
        ):
            wq_sb = const_pool.tile([128, NCH, H], BF16)
            wk_sb = const_pool.tile([128, NCH, H], BF16)
            wv_sb = const_pool.tile([128, NCH, H], BF16)
            nc.sync.dma_start(wq_sb[:], wqT.rearrange("(c p) h -> p c h", p=128))
            nc.sync.dma_start(wk_sb[:], wkT.rearrange("(c p) h -> p c h", p=128))
            nc.sync.dma_start(wv_sb[:], wvT.rearrange("(c p) h -> p c h", p=128))
            mask_sb = const_pool.tile([128, 4, SBLK], BF16)
            nc.sync.dma_start(mask_sb[:], masks.rearrange("m p q -> p m q"))
            rs = const_pool.tile([128, 2], F32)
            nc.sync.dma_start(rs[:], rolesel[:])
            biasln = const_pool.tile([128, 1], F32)
            nc.vector.memset(biasln[:], -LN16)
            ones8 = const_pool.tile([128, 2, H], E4)
            nc.vector.memset(ones8[:], 1.0)
            onesr8 = const_pool.tile([128, 2, H], E4)
            nc.vector.tensor_scalar_mul(onesr8[:], ones8[:], rs[:, 0:1])

            qt_bufs = [big_pool.tile([128, QLOC], BF16, name=f"qt{i}") for i in range(2)]
            ktl_bufs = [big_pool.tile([128, QLOC], BF16, name=f"ktl{i}") for i in range(2)]
            ktp_bufs = [big_pool.tile([128, QLOC], BF16, name=f"ktp{i}") for i in range(2)]
            vloc_bufs = [
                big_pool.tile([128, NVB, H + 1], BF16, name=f"vloc{i}") for i in range(2)
            ]
            vpa8_bufs = [
                big_pool.tile([128, NVB, H], E4, name=f"vpa8{i}") for i in range(2)
            ]
            vpd8_bufs = [
                big_pool.tile([128, NVB, H], E4, name=f"vpd8{i}") for i in range(2)
            ]
            vpa16_bufs = [
                big_pool.tile([128, NVB, H + 1], BF16, name=f"vpa16{i}")
                for i in range(2)
            ]
            vpd16_bufs = [
                big_pool.tile([128, NVB, H + 1], BF16, name=f"vpd16{i}")
                for i in range(2)
            ]
            st_bufs = [
                big_pool.tile([128, 2, 3 * 1024], BF16, name=f"st{i}") for i in range(2)
            ]
            recv_bufs = [
                big_pool.tile([128, 3 * 1024], BF16, name=f"recv{i}") for i in range(2)
            ]

            for vb in vloc_bufs + vpa16_bufs:
                nc.vector.memset(vb[:, :, H], 1.0)

            const_pt = const_pool.tile([128, 2, SBLK], BF16)
            nc.vector.memset(const_pt[:], 0.001)

            def load_pair_e3(src_dram, s_off, tag):
                strip = strip_pool.tile([128, NCH, 2 * SBLK], E3, tag=tag)
                nc.sync.dma_start(
                    strip[:],
                    src_dram[:, s_off : s_off + 2 * SBLK].rearrange(
                        "(c p) s -> p c s", p=128
                    ),
                )
                return strip

            def load_single_bf(src_dram, s_off, tag):
                strip = strip_pool.tile([128, NCH, SBLK], BF16, tag=tag)
                nc.sync.dma_start(
                    strip[:],
                    src_dram[:, s_off : s_off + SBLK].rearrange(
                        "(c p) s -> p c s", p=128
                    ),
                )
                return strip

            def project_qk(dst, w_sb, src_dram, t, tag, dup_eng, e3src):
                """col-paired projection of strips (2t, 2t+1)."""
                a, b2 = 2 * t, 2 * t + 1
                if e3src:
                    sp = load_pair_e3(src_dram, a * SBLK, tag)
                    sa = sp[:, :, 0:SBLK]
                    sb = sp[:, :, SBLK : 2 * SBLK]
                else:
                    sa = load_single_bf(src_dram, a * SBLK, tag)[:]
                    sb = load_single_bf(src_dram, b2 * SBLK, tag)[:]
                ps = pp.tile([128, SBLK], F32, tag="proj")
                for c in range(NCH):
                    nc.tensor.matmul(
                        ps[0:64, :], w_sb[:, c, :], sa[:, c, :],
                        start=(c == 0), stop=(c == NCH - 1),
                        skip_group_check=True,
                    )
                    nc.tensor.matmul(
                        ps[64:128, :], w_sb[:, c, :], sb[:, c, :],
                        start=(c == 0), stop=(c == NCH - 1),
                        skip_group_check=True,
                    )
                nc.vector.tensor_copy(
                    dst[0:64, a * SBLK : (a + 1) * SBLK], ps[0:64, :]
                )
                nc.vector.tensor_copy(
                    dst[64:128, b2 * SBLK : (b2 + 1) * SBLK], ps[64:128, :]
                )
                dup_eng.dma_start(
                    dst[64:128, a * SBLK : (a + 1) * SBLK],
                    dst[0:64, a * SBLK : (a + 1) * SBLK],
                )
                dup_eng.dma_start(
                    dst[0:64, b2 * SBLK : (b2 + 1) * SBLK],
                    dst[64:128, b2 * SBLK : (b2 + 1) * SBLK],
                )

            def project_vT(dst_vloc, t):
                """v-strip-pair chunks as lhsT: two psum [128, 4, H] tiles."""
                sp = load_pair_e3(vT, 2 * t * SBLK, "vstrip")
                for half in range(2):
                    psv = pp.tile([128, 4, H], F32, tag="proj")
                    for j in range(4):
                        jo = half * 4 + j
                        for c in range(NCH):
                            nc.tensor.matmul(
                                psv[:, j, :],
                                sp[:, c, jo * 128 : (jo + 1) * 128],
                                wv_sb[:, c, :],
                                start=(c == 0), stop=(c == NCH - 1),
                                skip_group_check=True,
                            )
                    g0 = 8 * t + 4 * half
                    nc.vector.tensor_copy(dst_vloc[:, g0 : g0 + 4, :H], psv[:])

            def project_kv_and_cc(nxt):
                ktl = ktl_bufs[nxt]
                vloc = vloc_bufs[nxt]
                for t in range(NSLOT // 2):
                    project_qk(ktl, wk_sb, kT, t, "kstrip", nc.gpsimd, False)
                for t in range(NSLOT // 2):
                    project_vT(vloc, t)
                st = st_bufs[nxt]
                for r in range(2):
                    sc_r = rs[:, r : r + 1]
                    nc.vector.tensor_scalar_mul(
                        st[0:64, r, 0:1024], ktl[0:64, 0:1024], sc_r[0:64]
                    )
                    nc.vector.tensor_scalar_mul(
                        st[64:128, r, 0:1024], ktl[64:128, 1024:2048], sc_r[64:128]
                    )
                    nc.vector.tensor_scalar_mul(
                        st[:, r, 1024 : 1024 + NVB * H].rearrange(
                            "p (b h) -> p b h", b=NVB
                        ),
                        vloc[:, :, :H],
                        sc_r,
                    )
                cc_in = dram_pool.tile([2, 128, 3 * 1024], BF16, tag=f"cc_in{nxt}")
                cc_out = dram_pool.tile([128, 3 * 1024], BF16, tag=f"cc_out{nxt}")
                nc.gpsimd.dma_start(cc_in[:].rearrange("a p c -> p a c"), st[:])
                nc.gpsimd.collective_compute(
                    "ReduceScatter", mybir.AluOpType.add,
                    replica_groups=RG_PAIRS,
                    ins=[cc_in.opt()], outs=[cc_out.opt()],
                )
                return cc_out

            def combine(par, cc_out):
                recv = recv_bufs[par]
                nc.gpsimd.dma_start(recv[:], cc_out[:])
                ktp = ktp_bufs[par]
                nc.gpsimd.dma_start(ktp[0:64, 0:1024], recv[0:64, 0:1024])
                nc.gpsimd.dma_start(ktp[0:64, 1024:2048], recv[64:128, 0:1024])
                nc.gpsimd.dma_start(ktp[64:128, :], ktp[0:64, :])
                vrecv = recv[:, 1024 : 1024 + NVB * H].rearrange(
                    "p (b h) -> p b h", b=NVB
                )
                nc.gpsimd.tensor_copy(vpa8_bufs[par][:], vrecv)
                nc.gpsimd.tensor_scalar_mul(
                    vpd8_bufs[par][:], vpa8_bufs[par][:], rs[:, 0:1]
                )
                nc.gpsimd.tensor_copy(vpa16_bufs[par][:, :, :H], vrecv)
                nc.gpsimd.tensor_scalar_mul(
                    vpd16_bufs[par][:], vpa16_bufs[par][:], rs[:, 0:1]
                )

            def attn_pass(par):
                qt_sb = qt_bufs[par]
                kt_loc, kt_peer = ktl_bufs[par], ktp_bufs[par]
                vloc = vloc_bufs[par]
                vpa8, vpd8 = vpa8_bufs[par], vpd8_bufs[par]
                vpa16, vpd16 = vpa16_bufs[par], vpd16_bufs[par]
                exp_ctr = [0]

                def scores_pair(kt, p, s, trim=0):
                    w = SBLK - trim
                    ps2 = ps_sc.tile([128, 2, SBLK], F32, tag="scores")
                    j0, j1 = 2 * p, 2 * p + 1
                    qs = slice(s * SBLK + trim, (s + 1) * SBLK)
                    nc.tensor.matmul(
                        ps2[:, 0, :w], kt[0:64, j0 * 128 : (j0 + 1) * 128],
                        qt_sb[0:64, qs], start=True, stop=True,
                        skip_group_check=True,
                    )
                    nc.tensor.matmul(
                        ps2[:, 1, :w], kt[64:128, j1 * 128 : (j1 + 1) * 128],
                        qt_sb[64:128, qs], start=True, stop=True,
                        skip_group_check=True,
                    )
                    return ps2, w

                def attn_pair_bf16(po, kt, p, va, s, mask2, first, trim=0,
                                   sch=False):
                    ps2, w = scores_pair(kt, p, s, trim)
                    j0, j1 = 2 * p, 2 * p + 1
                    if phase == "noexp":
                        pt2 = const_pt[:]
                    elif sch:
                        pti = work_pool.tile([128, 2, SBLK], I16, tag="pt16")
                        nc.vector.tensor_scalar(
                            pti[:], ps2[:], A16, B16,
                            mybir.AluOpType.mult, mybir.AluOpType.add,
                        )
                        pt2 = pti[:].bitcast(BF16)
                    else:
                        pt2t = work_pool.tile([128, 2, SBLK], BF16, tag="pt16")
                        nc.scalar.activation(
                            pt2t[:, :, :w], ps2[:, :, :w],
                            mybir.ActivationFunctionType.Exp,
                            bias=biasln[:], scale=SCALE,
                        )
                        if mask2 is not None:
                            nc.vector.tensor_mul(
                                pt2t[:, :, :w], pt2t[:, :, :w], mask2
                            )
                        pt2 = pt2t[:]
                    nc.tensor.matmul(
                        po[:, trim:], va[:, j0, :], pt2[:, 0, :w],
                        start=first, stop=False, skip_group_check=True,
                    )
                    nc.tensor.matmul(
                        po[:, trim:], va[:, j1, :], pt2[:, 1, :w],
                        start=False, stop=False, skip_group_check=True,
                    )

                def attn_pair_fp8(po, dn, kt, p, va8, s, firstd, last,
                                  diag_den=False):
                    ps2, w = scores_pair(kt, p, s, 0)
                    j0 = 2 * p
                    pt8t = work_pool.tile([128, 2, SBLK], E4, tag="pt8")
                    if phase == "noexp":
                        nc.vector.tensor_copy(pt8t[:], const_pt[:])
                    else:
                        nc.scalar.activation(
                            pt8t[:], ps2[:],
                            mybir.ActivationFunctionType.Exp,
                            bias=biasln[:], scale=SCALE,
                        )
                    pt8 = pt8t[:]
                    nc.tensor.matmul(
                        po[0:64, :], va8[:, j0 : j0 + 2, :], pt8,
                        start=False, stop=last,
                        perf_mode=DR, skip_group_check=True,
                    )
                    ones_t = onesr8 if diag_den else ones8
                    nc.tensor.matmul(
                        dn[:], ones_t[:], pt8,
                        start=firstd, stop=last,
                        perf_mode=DR, skip_group_check=True,
                    )

                def full_pair(po, dn, kt, p, s, kind, firstd, last):
                    use_dve = exp_ctr[0] % dve_every == (dve_every - 1)
                    exp_ctr[0] += 1
                    va16 = {
                        "loc": vloc, "peer": vpa16, "peerdiag": vpd16
                    }[kind]
                    if use_dve and phase != "noexp":
                        attn_pair_bf16(
                            po, kt, p, va16, s, None, first=False, sch=True
                        )
                        return firstd
                    if kind == "loc":
                        attn_pair_bf16(
                            po, kt, p, va16, s, None, first=False, sch=False
                        )
                        return firstd
                    va8 = {"peer": vpa8, "peerdiag": vpd8}[kind]
                    attn_pair_fp8(
                        po, dn, kt, p, va8, s, firstd, last,
                        diag_den=(kind == "peerdiag"),
                    )
                    return False

                for s in range(NSLOT):
                    po = ps_o.tile([H + 1, SBLK], F32, tag="oT")
                    dn = ps_dn.tile([H, SBLK], F32, tag="dn")
                    for h2 in range(2):
                        trim = 256 * h2
                        mask2 = mask_sb[:, 2 * h2 : 2 * h2 + 2, trim:]
                        attn_pair_bf16(
                            po, kt_loc, 2 * s + h2, vloc, s, mask2,
                            first=(h2 == 0), trim=trim,
                        )
                    firstd = True
                    nfull = 2 * s + 2 * (s + 1)
                    ifull = 0
                    for g in range(s):
                        for h2 in range(2):
                            ifull += 1
                            firstd = full_pair(
                                po, dn, kt_loc, 2 * g + h2, s, "loc",
                                firstd, last=(ifull == nfull),
                            )
                    for g in range(s + 1):
                        diag = g == s
                        for h2 in range(2):
                            ifull += 1
                            firstd = full_pair(
                                po, dn, kt_peer, 2 * g + h2, s,
                                "peerdiag" if diag else "peer",
                                firstd, last=(ifull == nfull),
                            )
                    if firstd:
                        nc.vector.memset(dn[0:1, :], 0.0)
                    cols = slice(s * SBLK, (s + 1) * SBLK)
                    ot_t = owork_pool.tile([H + 1, SBLK], F32, tag="ot")
                    dn_t = owork_pool.tile([1, SBLK], F32, tag="dnsb")
                    nc.vector.tensor_copy(ot_t[:], po[:])
                    nc.vector.tensor_copy(dn_t[:], dn[0:1, :])
                    nc.gpsimd.dma_start(out[0 : H + 1, cols], ot_t[:])
                    nc.gpsimd.dma_start(out[H + 1 : H + 2, cols], dn_t[:])

            if phase == "dma":
                dummy = const_pool.tile([1, QLOC], F32)
                for _rep in range(repeat):
                    for g in range(NSLOT // 2):
                        s1 = load_pair_e3(qT, 2 * g * SBLK, "qstrip")
                        s2 = load_pair_e3(vT, 2 * g * SBLK, "vstrip")
                        s3 = load_single_bf(kT, 2 * g * SBLK, "kstrip")
                        s4 = load_single_bf(kT, (2 * g + 1) * SBLK, "kstrip")
                        nc.vector.tensor_copy(dummy[:1, :2], s1[:1, 0, :2])
                        nc.vector.tensor_copy(dummy[:1, 2:4], s2[:1, 0, :2])
                        nc.vector.tensor_copy(dummy[:1, 4:6], s3[:1, 0, :2])
                        nc.vector.tensor_copy(dummy[:1, 6:8], s4[:1, 0, :2])
                    nc.vector.memset(dummy[:1, :], 0.0)
                    nc.sync.dma_start(out[:1, :], dummy[:1, :])
            else:
                cc_outs = {0: project_kv_and_cc(0)}
                for i in range(repeat):
                    par, nxt = i % 2, (i + 1) % 2
                    for t in range(NSLOT // 2):
                        project_qk(
                            qt_bufs[par], wq_sb, qT, t, "qstrip", nc.scalar, True
                        )
                    combine(par, cc_outs[par])
                    if i + 1 < repeat:
                        cc_outs[nxt] = project_kv_and_cc(nxt)
                    attn_pass(par)

    nc.compile()
    return nc


def fold_rows(r):
    return np.concatenate(
        [np.arange(512 * (2 * s + r), 512 * (2 * s + r) + 512) for s in range(4)]
    )


def make_in_maps(q, k, v, Wq, Wk, Wv, mode=None):
    wqT = np.ascontiguousarray(Wq.T * 32.0).astype(ml_dtypes.bfloat16)
    wkT = np.ascontiguousarray(Wk.T * 32.0).astype(ml_dtypes.bfloat16)
    wvT = np.ascontiguousarray(Wv.T * 32.0).astype(ml_dtypes.bfloat16)

    kk = np.arange(128)[:, None]
    qq = np.arange(SBLK)[None, :]
    masks = np.stack(
        [(qq >= kk + 128 * m).astype(ml_dtypes.bfloat16) for m in range(4)]
    )

    in_maps = []
    for c in range(8):
        b, r = c // 2, c % 2
        rows = fold_rows(r)
        rsel = np.zeros((128, 2), dtype=np.float32)
        rsel[:, 0] = 1.0 if r == 1 else 0.0
        rsel[:, 1] = 1.0 if r == 0 else 0.0
        in_maps.append(
            {
                "qT": np.ascontiguousarray(q[b][rows].T).astype(
                    ml_dtypes.float8_e3m4
                ),
                "kT": np.ascontiguousarray(k[b][rows].T).astype(ml_dtypes.bfloat16),
                "vT": np.ascontiguousarray(v[b][rows].T).astype(
                    ml_dtypes.float8_e3m4
                ),
                "wqT": wqT,
                "wkT": wkT,
                "wvT": wvT,
                "masks": masks,
                "rolesel": rsel,
            }
        )
    return in_maps


def assemble_output(results):
    out = np.zeros((B, S, H), dtype=np.float32)
    for c in range(8):
        b, r = c // 2, c % 2
        oT = np.asarray(results[c]["oT"], dtype=np.float32)
        for s in range(4):
            num = oT[:H, s * SBLK : (s + 1) * SBLK]
            den = (
                oT[H, s * SBLK : (s + 1) * SBLK]
                + oT[H + 1, s * SBLK : (s + 1) * SBLK]
            )
            g = 512 * (2 * s + r)
            out[b, g : g + 512, :] = (num / (32.0 * den[None, :])).T
    return out


def host_rescue(out, q, k, v, Wq, Wk, Wv):
    bad_b, bad_i = np.where(~np.isfinite(out).all(axis=2))
    if len(bad_b) == 0:
        return out
    scale = 1.0 / np.sqrt(np.float32(H))
    for b in np.unique(bad_b):
        idx = bad_i[bad_b == b]
        K_ = k[b] @ Wk.T
        V_ = v[b] @ Wv.T
        for i in idx:
            qr = q[b, i] @ Wq.T
            s = (K_[: i + 1] @ qr) * scale
            p = np.exp(s - s.max())
            out[b, i] = (p @ V_[: i + 1]) / p.sum()
    return out


_NC_CACHE = {}


def kernel(q, k, v, Wq, Wk, Wv):
    q = np.asarray(q, dtype=np.float32)
    k = np.asarray(k, dtype=np.float32)
    v = np.asarray(v, dtype=np.float32)
    Wq = np.asarray(Wq, dtype=np.float32)
    Wk = np.asarray(Wk, dtype=np.float32)
    Wv = np.asarray(Wv, dtype=np.float32)

    if "nc" not in _NC_CACHE:
        _NC_CACHE["nc"] = build_kernel()
    nc = _NC_CACHE["nc"]
    in_maps = make_in_maps(q, k, v, Wq, Wk, Wv)
    last_exc = None
    for attempt in range(3):
        try:
            res = run_bass_kernel_spmd(nc, in_maps, core_ids=list(range(8)))
            out = assemble_output(res.results)
            return host_rescue(out, q, k, v, Wq, Wk, Wv)
        except Exception as e:
            last_exc = e
            import time as _time

            _time.sleep(15 * (attempt + 1))
    raise last_exc
